# revision 13
# baseline (speedup 1.0000x reference)
"""Trainium2 Bass kernel for nn_D3PMCrossAttention (encoder-decoder diffusion
transformer, D=512 H=8 DF=2048 NL=6+6 V=16000 L=512 B=8).

Strategy: pure data-parallel over batch (1 element per NeuronCore, 8 cores,
no collectives). Activations live transposed in SBUF as [D(partitions), L(free)]
so every linear is matmul(lhsT=W[d_in,d_out], rhs=X^T) with weights used as
stored.

Precision: every matmul runs in float32r - full fp32 data processed at
1 cycle/row on the PE (same rate as bf16 for moving dim >= 256), ~1.5e-4
per-matmul error. A single PE dtype also avoids the fp32r/bf16 mode-switch
hazard (alternating dtypes was observed to corrupt results and even hang the
PE). PSUM accumulation, layernorm and softmax arithmetic are fp32. This is
deliberately the memory-bound regime: ~245 MB of fp32 weights stream per
core per call, overlapped with compute.

Tricks:
- softmax: scores built transposed [k, q] so the padding-mask bias is a
  per-partition scalar folded into the Exp activation; denominators come from
  a 65th all-ones column appended to V in the attn@V matmul; the reciprocal
  row is broadcast across partitions on the (otherwise idle) GPSIMD.
- layernorm: sum/sumsq over the feature dim (partitions) via an all-ones
  128x128 lhsT matmul, which reduces AND broadcasts to all partitions in one
  PE op.
- 1/sqrt(HD) pre-folded into Wq on the host (exact: x0.125).
- all linear biases / ln g,b / head_b are structurally zero/one in this
  problem (asserted on host) and skipped on device.
"""
import numpy as np

D, H, DF, NL, V, LMAX, B = 512, 8, 2048, 6, 16000, 512, 8
T_MAX, MASK_ID, COS_S = 1000, 4, 0.008
HD = D // H          # 64
P = 128
KO = D // P          # 4 chunks of the feature dim
KF = DF // P         # 16 chunks of the ff dim
L = LMAX
NEG = -60.0          # mask bias (exp(-60) ~ 9e-27, close enough to -inf)

_MODULE_CACHE = {}


def _build_module():
    import concourse.bacc as bacc
    import concourse.mybir as mybir
    import concourse.tile as tile

    FP32 = mybir.dt.float32
    F32R = mybir.dt.float32r
    AF = mybir.ActivationFunctionType

    nc = bacc.Bacc("TRN2", target_bir_lowering=False, debug=False, num_devices=8)

    # ---------------- DRAM I/O ----------------
    x0e_d = nc.dram_tensor("x0e", [D, L], F32R, kind="ExternalInput")
    x0d_d = nc.dram_tensor("x0d", [D, L], F32R, kind="ExternalInput")
    ewq_d = nc.dram_tensor("ew_qkvo", [NL, 4, D, D], F32R, kind="ExternalInput")
    ewf1_d = nc.dram_tensor("ew_f1", [NL, D, DF], F32R, kind="ExternalInput")
    ewf2_d = nc.dram_tensor("ew_f2", [NL, DF, D], F32R, kind="ExternalInput")
    dwq_d = nc.dram_tensor("dw_qkvo", [NL, 8, D, D], F32R, kind="ExternalInput")
    dwf1_d = nc.dram_tensor("dw_f1", [NL, D, DF], F32R, kind="ExternalInput")
    dwf2_d = nc.dram_tensor("dw_f2", [NL, DF, D], F32R, kind="ExternalInput")
    tokT_d = nc.dram_tensor("tokT", [D, V], F32R, kind="ExternalInput")
    vecs_d = nc.dram_tensor("vecs", [P, 8], FP32, kind="ExternalInput")
    out_d = nc.dram_tensor("logitsT", [V, L], FP32, kind="ExternalOutput")

    def r3(ap):  # [D_in, N] dram -> [128, ko, N]
        return ap.rearrange("(ko p) n -> p ko n", p=P)

    with tile.TileContext(nc) as tc:
        with tc.tile_pool(name="const", bufs=1) as cst, \
             tc.tile_pool(name="w", bufs=2) as wp, \
             tc.tile_pool(name="act", bufs=2) as ap_, \
             tc.tile_pool(name="ps", bufs=2, space="PSUM") as ps:

            # constants
            onesf = cst.tile([P, P], FP32)
            nc.vector.memset(onesf[:], 1.0)
            ones_r = cst.tile([P, P], F32R)
            nc.vector.tensor_copy(ones_r[:], onesf[:])
            zcol = cst.tile([P, 2], FP32)
            nc.vector.memset(zcol[:, 0:1], 0.0)
            nc.vector.memset(zcol[:, 1:2], 1e-5)
            bias0 = zcol[:, 0:1]
            biaseps = zcol[:, 1:2]
            vecs = cst.tile([P, 8], FP32)
            nc.sync.dma_start(vecs[:], vecs_d.ap())
            src_bias = vecs[:, 0:4]   # [128, 4] per k-chunk columns
            tgt_bias = vecs[:, 4:8]

            def load_w(dram_ap, tag, bufs):
                t = wp.tile([P, KO, D], F32R, tag=tag, bufs=bufs)
                nc.sync.dma_start(t[:], r3(dram_ap))
                return t

            def ln_inplace(X):
                """X (f32r [P,KO,L]) -> Xn (f32r) layernormed over d."""
                Xsq = ap_.tile([P, KO, L], F32R, tag="Xsq", bufs=1)
                nc.scalar.activation(Xsq[:], X[:], AF.Square, bias=bias0)
                pS = ps.tile([P, L], FP32, tag="psA")
                pSS = ps.tile([P, L], FP32, tag="psA")
                for ko in range(KO):
                    nc.tensor.matmul(pS[:], ones_r[:], X[:, ko, :],
                                     start=(ko == 0), stop=(ko == KO - 1))
                for ko in range(KO):
                    nc.tensor.matmul(pSS[:], ones_r[:], Xsq[:, ko, :],
                                     start=(ko == 0), stop=(ko == KO - 1))
                m_b = ap_.tile([P, L], FP32, tag="stat", bufs=6)
                nc.vector.tensor_scalar_mul(m_b[:], pS[:], 1.0 / D)
                msq = ap_.tile([P, L], FP32, tag="stat", bufs=6)
                nc.scalar.activation(msq[:], m_b[:], AF.Square, bias=bias0)
                var = ap_.tile([P, L], FP32, tag="stat", bufs=6)
                nc.vector.tensor_scalar(var[:], pSS[:], 1.0 / D, None,
                                        mybir.AluOpType.mult)
                nc.vector.tensor_sub(var[:], var[:], msq[:])
                sd = ap_.tile([P, L], FP32, tag="stat", bufs=6)
                nc.scalar.activation(sd[:], var[:], AF.Sqrt, bias=biaseps)
                rstd = ap_.tile([P, L], FP32, tag="stat", bufs=6)
                nc.vector.reciprocal(rstd[:], sd[:])
                c_b = ap_.tile([P, L], FP32, tag="stat", bufs=6)
                nc.vector.tensor_mul(c_b[:], m_b[:], rstd[:])
                Xn = ap_.tile([P, KO, L], F32R, tag="X")
                rb = rstd[:, None, :].to_broadcast([P, KO, L])
                cb = c_b[:, None, :].to_broadcast([P, KO, L])
                nc.vector.tensor_mul(Xn[:], X[:], rb)
                nc.vector.tensor_sub(Xn[:], Xn[:], cb)
                return Xn

            def attention(Xq, KV, wq, wk, wv, wo, mask_cols, X_res):
                """MHA block, all f32r. Returns X_res + attn_out (f32r)."""
                QT = ap_.tile([P, KO, L], F32R, tag="QT", bufs=1)
                KT = ap_.tile([P, KO, L], F32R, tag="KT", bufs=1)
                for dst, w, src_ in ((QT, wq, Xq), (KT, wk, KV)):
                    for mo in range(KO):
                        pst = ps.tile([P, L], FP32, tag="proj", bufs=4)
                        for ko in range(KO):
                            nc.tensor.matmul(pst[:], w[:, ko, mo * P:(mo + 1) * P],
                                             src_[:, ko, :], start=(ko == 0),
                                             stop=(ko == KO - 1))
                        nc.any.tensor_copy(dst[:, mo, :], pst[:])
                # V in normal layout [k, d] with per-head ones column (65-wide)
                Vpp = ap_.tile([P, KO, H * 65], F32R, tag="Vpp", bufs=1)
                nc.vector.tensor_copy(
                    Vpp[:].rearrange("p m (h c) -> p m h c", c=65)[:, :, :, 64],
                    onesf[:, None, 0:8].to_broadcast([P, KO, 8]))
                for mo in range(KO):
                    pst = ps.tile([P, L], FP32, tag="proj", bufs=4)
                    for ko in range(KO):
                        nc.tensor.matmul(pst[:], KV[:, ko, mo * P:(mo + 1) * P],
                                         wv[:, ko, :], start=(ko == 0), stop=(ko == KO - 1))
                    nc.any.tensor_copy(
                        Vpp[:, mo, :].rearrange("p (h c) -> p h c", c=65)[:, :, 0:64],
                        pst[:])
                # per-head attention
                AOT = ap_.tile([P, KO, L], F32R, tag="AOT", bufs=1)
                for h in range(H):
                    koh, p0 = h // 2, (h % 2) * 64
                    pso = ps.tile([65, L], FP32, tag="attnv")
                    for kc in range(KO):
                        pss = ps.tile([P, L], FP32, tag="psA")
                        nc.tensor.matmul(pss[:],
                                         KT[p0:p0 + 64, koh, kc * P:(kc + 1) * P],
                                         QT[p0:p0 + 64, koh, :], start=True, stop=True)
                        expT = ap_.tile([P, L], F32R, tag="expT", bufs=4)
                        nc.scalar.activation(expT[:], pss[:], AF.Exp,
                                             bias=mask_cols[:, kc:kc + 1])
                        nc.tensor.matmul(pso[:],
                                         Vpp[:, kc, h * 65:(h + 1) * 65],
                                         expT[:], start=(kc == 0), stop=(kc == KO - 1))
                    r_sb = ap_.tile([1, L], FP32, tag="stat", bufs=6)
                    nc.vector.reciprocal(r_sb[:], pso[64:65, :])
                    rbc = ap_.tile([64, L], FP32, tag="rbc")
                    nc.gpsimd.partition_broadcast(rbc[:], r_sb[:])
                    nc.vector.tensor_mul(AOT[p0:p0 + 64, koh, :], pso[0:64, :], rbc[:])
                # output projection + residual
                Xn = ap_.tile([P, KO, L], F32R, tag="X")
                for mo in range(KO):
                    pst = ps.tile([P, L], FP32, tag="proj", bufs=4)
                    for ko in range(KO):
                        nc.tensor.matmul(pst[:], wo[:, ko, mo * P:(mo + 1) * P],
                                         AOT[:, ko, :], start=(ko == 0), stop=(ko == KO - 1))
                    nc.vector.tensor_add(Xn[:, mo, :], pst[:], X_res[:, mo, :])
                return Xn

            def ffn(Xn, f1_ap, f2_ap, X_res):
                """f32r FFN with d_ff processed in 4 streamed quarters."""
                psums = [ps.tile([P, L], FP32, tag="proj", bufs=4, name=f"ffn_ps{_m}")
                         for _m in range(KO)]
                for qd in range(4):
                    w1q = wp.tile([P, KO, 512], F32R, tag="w1q", bufs=2)
                    nc.sync.dma_start(w1q[:], f1_ap[:, qd * 512:(qd + 1) * 512]
                                      .rearrange("(ko p) n -> p ko n", p=P))
                    w2q = wp.tile([P, KO, D], F32R, tag="w2q", bufs=2)
                    nc.sync.dma_start(w2q[:], f2_ap[qd * 512:(qd + 1) * 512, :]
                                      .rearrange("(ko p) n -> p ko n", p=P))
                    HTq = ap_.tile([P, KO, L], F32R, tag="HTq")
                    for m2 in range(KO):
                        pst = ps.tile([P, L], FP32, tag="psA")
                        for ko in range(KO):
                            nc.tensor.matmul(pst[:], w1q[:, ko, m2 * P:(m2 + 1) * P],
                                             Xn[:, ko, :], start=(ko == 0), stop=(ko == KO - 1))
                        nc.scalar.activation(HTq[:, m2, :], pst[:], AF.Relu, bias=bias0)
                    for mo in range(KO):
                        for kq in range(KO):
                            nc.tensor.matmul(psums[mo][:],
                                             w2q[:, kq, mo * P:(mo + 1) * P],
                                             HTq[:, kq, :],
                                             start=(qd == 0 and kq == 0),
                                             stop=(qd == 3 and kq == KO - 1))
                Xo = ap_.tile([P, KO, L], F32R, tag="X")
                for mo in range(KO):
                    nc.vector.tensor_add(Xo[:, mo, :], psums[mo][:], X_res[:, mo, :])
                return Xo

            # ---------------- encoder ----------------
            X = ap_.tile([P, KO, L], F32R, tag="X")
            nc.sync.dma_start(X[:], r3(x0e_d.ap()))
            for i in range(NL):
                wq = load_w(ewq_d.ap()[i, 0], "wqkv", 3)
                wk = load_w(ewq_d.ap()[i, 1], "wqkv", 3)
                wv = load_w(ewq_d.ap()[i, 2], "wqkv", 3)
                wo = load_w(ewq_d.ap()[i, 3], "wo", 2)
                X = attention(X, X, wq, wk, wv, wo, src_bias, X)
                X = ln_inplace(X)
                X = ffn(X, ewf1_d.ap()[i], ewf2_d.ap()[i], X)
                X = ln_inplace(X)
            memb = cst.tile([P, KO, L], F32R, tag="memb")
            nc.vector.tensor_copy(memb[:], X[:])

            # ---------------- decoder ----------------
            X = ap_.tile([P, KO, L], F32R, tag="X")
            nc.sync.dma_start(X[:], r3(x0d_d.ap()))
            for i in range(NL):
                wq = load_w(dwq_d.ap()[i, 0], "wqkv", 3)
                wk = load_w(dwq_d.ap()[i, 1], "wqkv", 3)
                wv = load_w(dwq_d.ap()[i, 2], "wqkv", 3)
                wo = load_w(dwq_d.ap()[i, 3], "wo", 2)
                X = attention(X, X, wq, wk, wv, wo, tgt_bias, X)
                X = ln_inplace(X)
                wq = load_w(dwq_d.ap()[i, 4], "wqkv", 3)
                wk = load_w(dwq_d.ap()[i, 5], "wqkv", 3)
                wv = load_w(dwq_d.ap()[i, 6], "wqkv", 3)
                wo = load_w(dwq_d.ap()[i, 7], "wo", 2)
                X = attention(X, memb, wq, wk, wv, wo, src_bias, X)
                X = ln_inplace(X)
                X = ffn(X, dwf1_d.ap()[i], dwf2_d.ap()[i], X)
                X = ln_inplace(X)

            # ---------------- tied vocab head ----------------
            vb = 0
            while vb < V:
                nb = min(256, V - vb)
                tkt = wp.tile([P, KO, 256], F32R, tag="tokT", bufs=3)
                nc.sync.dma_start(tkt[:, :, :nb], r3(tokT_d.ap()[:, vb:vb + nb]))
                for j in range(nb // P):
                    pst = ps.tile([P, L], FP32, tag="proj", bufs=4)
                    for ko in range(KO):
                        nc.tensor.matmul(pst[:], tkt[:, ko, j * P:(j + 1) * P],
                                         X[:, ko, :], start=(ko == 0), stop=(ko == KO - 1))
                    ot = ap_.tile([P, L], FP32, tag="lout")
                    nc.any.tensor_copy(ot[:], pst[:])
                    nc.sync.dma_start(out_d.ap()[vb + j * P: vb + (j + 1) * P, :], ot[:])
                vb += nb

    nc.compile()
    return nc


def _get_module():
    if "nc" not in _MODULE_CACHE:
        _MODULE_CACHE["nc"] = _build_module()
    return _MODULE_CACHE["nc"]


_QSAMPLE_SCRIPT = r"""
import sys
import numpy as np
import jax, jax.numpy as jnp
T_MAX, MASK_ID, COS_S = 1000, 4, 0.008
d = np.load(sys.argv[1])
t, tgt = d["t"], d["tgt"]
tf = jnp.asarray(t).astype(jnp.float32)
alpha_bar = jnp.cos(((tf / T_MAX) + COS_S) / (1.0 + COS_S) * (jnp.pi / 2)) ** 2
mask_prob = 1.0 - alpha_bar
u = jax.random.uniform(jax.random.key(42), tgt.shape)
x_t_ids = jnp.where(u < mask_prob[:, None], MASK_ID, jnp.asarray(tgt))
h = tf[:, None] @ jnp.asarray(d["t1w"]) + jnp.asarray(d["t1b"])
t_emb = jax.nn.silu(h) @ jnp.asarray(d["t2w"]) + jnp.asarray(d["t2b"])
np.savez(sys.argv[2], mask_prob=np.asarray(mask_prob),
         x_t_ids=np.asarray(x_t_ids), t_emb=np.asarray(t_emb))
"""


def _host_prep(src, tgt, t, params):
    """All cheap index/elementwise prep, replicating reference numerics.

    The q_sample randomness (jax threefry) is backend-dependent: the axon
    plugin returns different bits than XLA-CPU, so it MUST run in a CPU-only
    subprocess to match the reference."""
    import os
    import subprocess
    import sys
    import tempfile

    src = np.asarray(src)
    tgt = np.asarray(tgt)
    t = np.asarray(t)

    with tempfile.TemporaryDirectory() as td:
        fin = os.path.join(td, "in.npz")
        fout = os.path.join(td, "out.npz")
        np.savez(fin, t=t, tgt=tgt,
                 t1w=np.asarray(params["t1"]["W"]), t1b=np.asarray(params["t1"]["b"]),
                 t2w=np.asarray(params["t2"]["W"]), t2b=np.asarray(params["t2"]["b"]))
        env = dict(os.environ, JAX_PLATFORMS="cpu")
        subprocess.run([sys.executable, "-c", _QSAMPLE_SCRIPT, fin, fout],
                       check=True, env=env, capture_output=True)
        out = np.load(fout)
        mask_prob = out["mask_prob"]
        x_t_ids = out["x_t_ids"]
        t_emb = out["t_emb"]

    x_t_probs = np.broadcast_to(mask_prob[:, None], tgt.shape).astype(np.float32)

    src_tok = np.asarray(params["src_tok"])
    src_pos = np.asarray(params["src_pos"])
    tgt_tok = np.asarray(params["tgt_tok"])
    tgt_pos = np.asarray(params["tgt_pos"])

    def chk0(x):
        assert not np.any(np.asarray(x)), "expected zero vector"

    def chk1(x):
        assert np.all(np.asarray(x) == 1.0), "expected unit vector"

    def f32(x):
        return np.ascontiguousarray(np.asarray(x).astype(np.float32))

    qs = np.float32(1.0 / np.sqrt(HD))  # 0.125, exact
    ew_qkvo = np.empty((NL, 4, D, D), np.float32)
    ew_f1 = np.empty((NL, D, DF), np.float32)
    ew_f2 = np.empty((NL, DF, D), np.float32)
    for i, blk in enumerate(params["enc"]):
        m = blk["mha"]
        ew_qkvo[i, 0] = f32(np.asarray(m["q"]["W"]) * qs)
        ew_qkvo[i, 1] = f32(m["k"]["W"])
        ew_qkvo[i, 2] = f32(m["v"]["W"])
        ew_qkvo[i, 3] = f32(m["o"]["W"])
        ew_f1[i] = f32(blk["ff1"]["W"])
        ew_f2[i] = f32(blk["ff2"]["W"])
        for lin in (m["q"], m["k"], m["v"], m["o"], blk["ff1"], blk["ff2"]):
            chk0(lin["b"])
        for ln in (blk["ln1"], blk["ln2"]):
            chk1(ln["g"]); chk0(ln["b"])
    dw_qkvo = np.empty((NL, 8, D, D), np.float32)
    dw_f1 = np.empty((NL, D, DF), np.float32)
    dw_f2 = np.empty((NL, DF, D), np.float32)
    for i, blk in enumerate(params["dec"]):
        for j, m in enumerate((blk["sa"], blk["ca"])):
            dw_qkvo[i, 4 * j + 0] = f32(np.asarray(m["q"]["W"]) * qs)
            dw_qkvo[i, 4 * j + 1] = f32(m["k"]["W"])
            dw_qkvo[i, 4 * j + 2] = f32(m["v"]["W"])
            dw_qkvo[i, 4 * j + 3] = f32(m["o"]["W"])
            for lin in (m["q"], m["k"], m["v"], m["o"]):
                chk0(lin["b"])
        dw_f1[i] = f32(blk["ff1"]["W"])
        dw_f2[i] = f32(blk["ff2"]["W"])
        chk0(blk["ff1"]["b"]); chk0(blk["ff2"]["b"])
        for ln in (blk["ln1"], blk["ln2"], blk["ln3"]):
            chk1(ln["g"]); chk0(ln["b"])
    chk0(params["head_b"])
    tokT = f32(tgt_tok.T)

    in_maps = []
    for c in range(B):
        x0e = (src_tok[src[c]] + src_pos[:L]).T.astype(np.float32)
        x0d = (tgt_tok[x_t_ids[c]] + tgt_pos[:L] + t_emb[c][None, :]).T.astype(np.float32)
        vecs = np.zeros((P, 8), np.float32)
        sb = np.where(src[c] == 0, np.float32(NEG), np.float32(0.0))
        tb = np.where(tgt[c] == 0, np.float32(NEG), np.float32(0.0))
        vecs[:, 0:4] = sb.reshape(KO, P).T
        vecs[:, 4:8] = tb.reshape(KO, P).T
        in_maps.append({
            "x0e": np.ascontiguousarray(x0e),
            "x0d": np.ascontiguousarray(x0d),
            "ew_qkvo": ew_qkvo, "ew_f1": ew_f1, "ew_f2": ew_f2,
            "dw_qkvo": dw_qkvo, "dw_f1": dw_f1, "dw_f2": dw_f2,
            "tokT": tokT, "vecs": vecs,
        })
    return in_maps, x_t_probs


def kernel(src, tgt, t, params):
    from concourse.bass_utils import run_bass_kernel_spmd

    nc = _get_module()
    in_maps, x_t_probs = _host_prep(src, tgt, t, params)
    res = run_bass_kernel_spmd(nc, in_maps, core_ids=list(range(B)))
    logits = np.empty((B, L, V), np.float32)
    for c in range(B):
        logits[c] = res.results[c]["logitsT"].T
    return logits, x_t_probs


# revision 14
# speedup vs baseline: 1.1307x; 1.1307x over previous
"""Trainium2 Bass kernel for nn_D3PMCrossAttention (encoder-decoder diffusion
transformer, D=512 H=8 DF=2048 NL=6+6 V=16000 L=512 B=8).

Strategy: pure data-parallel over batch (1 element per NeuronCore, 8 cores,
no collectives). Activations live transposed in SBUF as [D(partitions), L(free)]
so every linear is matmul(lhsT=W[d_in,d_out], rhs=X^T) with weights used as
stored.

Precision: every matmul runs in float32r - full fp32 data processed at
1 cycle/row on the PE (same rate as bf16 for moving dim >= 256), ~1.5e-4
per-matmul error. A single PE dtype also avoids the fp32r/bf16 mode-switch
hazard (alternating dtypes was observed to corrupt results and even hang the
PE). PSUM accumulation, layernorm and softmax arithmetic are fp32. This is
deliberately the memory-bound regime: ~245 MB of fp32 weights stream per
core per call, overlapped with compute.

Tricks:
- softmax: scores built transposed [k, q] so the padding-mask bias is a
  per-partition scalar folded into the Exp activation; denominators come from
  a 65th all-ones column appended to V in the attn@V matmul; the reciprocal
  row is broadcast across partitions on the (otherwise idle) GPSIMD.
- layernorm: sum/sumsq over the feature dim (partitions) via an all-ones
  128x128 lhsT matmul, which reduces AND broadcasts to all partitions in one
  PE op.
- 1/sqrt(HD) pre-folded into Wq on the host (exact: x0.125).
- all linear biases / ln g,b / head_b are structurally zero/one in this
  problem (asserted on host) and skipped on device.
"""
import numpy as np

D, H, DF, NL, V, LMAX, B = 512, 8, 2048, 6, 16000, 512, 8
T_MAX, MASK_ID, COS_S = 1000, 4, 0.008
HD = D // H          # 64
P = 128
KO = D // P          # 4 chunks of the feature dim
KF = DF // P         # 16 chunks of the ff dim
L = LMAX
NEG = -60.0          # mask bias (exp(-60) ~ 9e-27, close enough to -inf)

_MODULE_CACHE = {}


def _build_module():
    import concourse.bacc as bacc
    import concourse.mybir as mybir
    import concourse.tile as tile

    FP32 = mybir.dt.float32
    F32R = mybir.dt.float32r
    AF = mybir.ActivationFunctionType

    nc = bacc.Bacc("TRN2", target_bir_lowering=False, debug=False, num_devices=8)

    # ---------------- DRAM I/O ----------------
    x0e_d = nc.dram_tensor("x0e", [D, L], F32R, kind="ExternalInput")
    x0d_d = nc.dram_tensor("x0d", [D, L], F32R, kind="ExternalInput")
    ewq_d = nc.dram_tensor("ew_qkvo", [NL, 4, D, D], F32R, kind="ExternalInput")
    ewf1_d = nc.dram_tensor("ew_f1", [NL, D, DF], F32R, kind="ExternalInput")
    ewf2_d = nc.dram_tensor("ew_f2", [NL, DF, D], F32R, kind="ExternalInput")
    dwq_d = nc.dram_tensor("dw_qkvo", [NL, 8, D, D], F32R, kind="ExternalInput")
    dwf1_d = nc.dram_tensor("dw_f1", [NL, D, DF], F32R, kind="ExternalInput")
    dwf2_d = nc.dram_tensor("dw_f2", [NL, DF, D], F32R, kind="ExternalInput")
    tokT_d = nc.dram_tensor("tokT", [D, V], F32R, kind="ExternalInput")
    vecs_d = nc.dram_tensor("vecs", [P, 8], FP32, kind="ExternalInput")
    out_d = nc.dram_tensor("logitsT", [V, L], FP32, kind="ExternalOutput")

    def r3(ap):  # [D_in, N] dram -> [128, ko, N]
        return ap.rearrange("(ko p) n -> p ko n", p=P)

    with tile.TileContext(nc) as tc:
        with tc.tile_pool(name="const", bufs=1) as cst, \
             tc.tile_pool(name="w", bufs=2) as wp, \
             tc.tile_pool(name="act", bufs=2) as ap_, \
             tc.tile_pool(name="ps", bufs=2, space="PSUM") as ps:

            # constants
            onesf = cst.tile([P, P], FP32)
            nc.vector.memset(onesf[:], 1.0)
            ones_r = cst.tile([P, P], F32R)
            nc.vector.tensor_copy(ones_r[:], onesf[:])
            zcol = cst.tile([P, 2], FP32)
            nc.vector.memset(zcol[:, 0:1], 0.0)
            nc.vector.memset(zcol[:, 1:2], 1e-5)
            bias0 = zcol[:, 0:1]
            biaseps = zcol[:, 1:2]
            vecs = cst.tile([P, 8], FP32)
            nc.sync.dma_start(vecs[:], vecs_d.ap())
            src_bias = vecs[:, 0:4]   # [128, 4] per k-chunk columns
            tgt_bias = vecs[:, 4:8]

            def load_w(dram_ap, tag, bufs):
                t = wp.tile([P, KO, D], F32R, tag=tag, bufs=bufs)
                nc.sync.dma_start(t[:], r3(dram_ap))
                return t

            def ln_inplace(X):
                """X (f32r [P,KO,L]) -> Xn (f32r) layernormed over d.

                Minimal serial chain: var*D = SS - S^2/D computed as
                TT(pSS - Square(pS * 1/sqrt(D))); sd = Sqrt(varD/D + eps)
                with the 1/D folded into the activation scale. The mean
                shift is applied as Xn = X*rstd - (pS*rstd)/D."""
                Xsq = ap_.tile([P, KO, L], F32R, tag="Xsq", bufs=1)
                for mo in range(KO):
                    nc.scalar.activation(Xsq[:, mo, :], X[:, mo, :], AF.Square,
                                         bias=bias0)
                pS = ps.tile([P, L], FP32, tag="psA")
                pSS = ps.tile([P, L], FP32, tag="psA")
                for ko in range(KO):
                    nc.tensor.matmul(pS[:], ones_r[:], X[:, ko, :],
                                     start=(ko == 0), stop=(ko == KO - 1))
                for ko in range(KO):
                    nc.tensor.matmul(pSS[:], ones_r[:], Xsq[:, ko, :],
                                     start=(ko == 0), stop=(ko == KO - 1))
                ps2 = ap_.tile([P, L], FP32, tag="stat", bufs=6)
                nc.scalar.activation(ps2[:], pS[:], AF.Square, bias=bias0,
                                     scale=float(1.0 / np.sqrt(D)))
                varD = ap_.tile([P, L], FP32, tag="stat", bufs=6)
                nc.vector.tensor_sub(varD[:], pSS[:], ps2[:])
                sd = ap_.tile([P, L], FP32, tag="stat", bufs=6)
                nc.scalar.activation(sd[:], varD[:], AF.Sqrt, bias=biaseps,
                                     scale=float(1.0 / D))
                rstd = ap_.tile([P, L], FP32, tag="stat", bufs=6)
                nc.vector.reciprocal(rstd[:], sd[:])
                cbD = ap_.tile([P, L], FP32, tag="stat", bufs=6)
                nc.vector.tensor_mul(cbD[:], pS[:], rstd[:])
                c_b = ap_.tile([P, L], FP32, tag="stat", bufs=6)
                nc.vector.tensor_scalar_mul(c_b[:], cbD[:], 1.0 / D)
                Xn = ap_.tile([P, KO, L], F32R, tag="X")
                for mo in range(KO):
                    nc.vector.tensor_mul(Xn[:, mo, :], X[:, mo, :], rstd[:])
                    nc.vector.tensor_sub(Xn[:, mo, :], Xn[:, mo, :], c_b[:])
                return Xn

            def attention(Xq, KV, wq, wk, wv, wo, mask_cols, X_res):
                """MHA block, all f32r. Returns X_res + attn_out (f32r)."""
                QT = ap_.tile([P, KO, L], F32R, tag="QT", bufs=1)
                KT = ap_.tile([P, KO, L], F32R, tag="KT", bufs=1)
                for dst, w, src_ in ((QT, wq, Xq), (KT, wk, KV)):
                    for mo in range(KO):
                        pst = ps.tile([P, L], FP32, tag="proj", bufs=4)
                        for ko in range(KO):
                            nc.tensor.matmul(pst[:], w[:, ko, mo * P:(mo + 1) * P],
                                             src_[:, ko, :], start=(ko == 0),
                                             stop=(ko == KO - 1))
                        nc.any.tensor_copy(dst[:, mo, :], pst[:])
                # V in normal layout [k, d] with per-head ones column (65-wide)
                Vpp = ap_.tile([P, KO, H * 65], F32R, tag="Vpp", bufs=1)
                nc.vector.tensor_copy(
                    Vpp[:].rearrange("p m (h c) -> p m h c", c=65)[:, :, :, 64],
                    onesf[:, None, 0:8].to_broadcast([P, KO, 8]))
                for mo in range(KO):
                    pst = ps.tile([P, L], FP32, tag="proj", bufs=4)
                    for ko in range(KO):
                        nc.tensor.matmul(pst[:], KV[:, ko, mo * P:(mo + 1) * P],
                                         wv[:, ko, :], start=(ko == 0), stop=(ko == KO - 1))
                    nc.any.tensor_copy(
                        Vpp[:, mo, :].rearrange("p (h c) -> p h c", c=65)[:, :, 0:64],
                        pst[:])
                # per-head attention
                AOT = ap_.tile([P, KO, L], F32R, tag="AOT", bufs=1)
                for h in range(H):
                    koh, p0 = h // 2, (h % 2) * 64
                    pso = ps.tile([65, L], FP32, tag="attnv")
                    for kc in range(KO):
                        pss = ps.tile([P, L], FP32, tag="psA")
                        nc.tensor.matmul(pss[:],
                                         KT[p0:p0 + 64, koh, kc * P:(kc + 1) * P],
                                         QT[p0:p0 + 64, koh, :], start=True, stop=True)
                        expT = ap_.tile([P, L], F32R, tag="expT", bufs=4)
                        nc.scalar.activation(expT[:], pss[:], AF.Exp,
                                             bias=mask_cols[:, kc:kc + 1])
                        nc.tensor.matmul(pso[:],
                                         Vpp[:, kc, h * 65:(h + 1) * 65],
                                         expT[:], start=(kc == 0), stop=(kc == KO - 1))
                    r_sb = ap_.tile([1, L], FP32, tag="stat", bufs=6)
                    nc.vector.reciprocal(r_sb[:], pso[64:65, :])
                    rbc = ap_.tile([64, L], FP32, tag="rbc")
                    nc.gpsimd.partition_broadcast(rbc[:], r_sb[:])
                    nc.vector.tensor_mul(AOT[p0:p0 + 64, koh, :], pso[0:64, :], rbc[:])
                # output projection + residual
                Xn = ap_.tile([P, KO, L], F32R, tag="X")
                for mo in range(KO):
                    pst = ps.tile([P, L], FP32, tag="proj", bufs=4)
                    for ko in range(KO):
                        nc.tensor.matmul(pst[:], wo[:, ko, mo * P:(mo + 1) * P],
                                         AOT[:, ko, :], start=(ko == 0), stop=(ko == KO - 1))
                    nc.vector.tensor_add(Xn[:, mo, :], pst[:], X_res[:, mo, :])
                return Xn

            def ffn(Xn, f1_ap, f2_ap, X_res):
                """f32r FFN with d_ff processed in 4 streamed quarters."""
                psums = [ps.tile([P, L], FP32, tag="proj", bufs=4, name=f"ffn_ps{_m}")
                         for _m in range(KO)]
                for qd in range(4):
                    w1q = wp.tile([P, KO, 512], F32R, tag="w1q", bufs=2)
                    nc.sync.dma_start(w1q[:], f1_ap[:, qd * 512:(qd + 1) * 512]
                                      .rearrange("(ko p) n -> p ko n", p=P))
                    w2q = wp.tile([P, KO, D], F32R, tag="w2q", bufs=2)
                    nc.sync.dma_start(w2q[:], f2_ap[qd * 512:(qd + 1) * 512, :]
                                      .rearrange("(ko p) n -> p ko n", p=P))
                    HTq = ap_.tile([P, KO, L], F32R, tag="HTq")
                    for m2 in range(KO):
                        pst = ps.tile([P, L], FP32, tag="psA")
                        for ko in range(KO):
                            nc.tensor.matmul(pst[:], w1q[:, ko, m2 * P:(m2 + 1) * P],
                                             Xn[:, ko, :], start=(ko == 0), stop=(ko == KO - 1))
                        nc.scalar.activation(HTq[:, m2, :], pst[:], AF.Relu, bias=bias0)
                    for mo in range(KO):
                        for kq in range(KO):
                            nc.tensor.matmul(psums[mo][:],
                                             w2q[:, kq, mo * P:(mo + 1) * P],
                                             HTq[:, kq, :],
                                             start=(qd == 0 and kq == 0),
                                             stop=(qd == 3 and kq == KO - 1))
                Xo = ap_.tile([P, KO, L], F32R, tag="X")
                for mo in range(KO):
                    nc.vector.tensor_add(Xo[:, mo, :], psums[mo][:], X_res[:, mo, :])
                return Xo

            # ---------------- encoder ----------------
            X = ap_.tile([P, KO, L], F32R, tag="X")
            nc.sync.dma_start(X[:], r3(x0e_d.ap()))
            for i in range(NL):
                wq = load_w(ewq_d.ap()[i, 0], "wqkv", 3)
                wk = load_w(ewq_d.ap()[i, 1], "wqkv", 3)
                wv = load_w(ewq_d.ap()[i, 2], "wqkv", 3)
                wo = load_w(ewq_d.ap()[i, 3], "wo", 2)
                X = attention(X, X, wq, wk, wv, wo, src_bias, X)
                X = ln_inplace(X)
                X = ffn(X, ewf1_d.ap()[i], ewf2_d.ap()[i], X)
                X = ln_inplace(X)
            memb = cst.tile([P, KO, L], F32R, tag="memb")
            nc.vector.tensor_copy(memb[:], X[:])

            # ---------------- decoder ----------------
            X = ap_.tile([P, KO, L], F32R, tag="X")
            nc.sync.dma_start(X[:], r3(x0d_d.ap()))
            for i in range(NL):
                wq = load_w(dwq_d.ap()[i, 0], "wqkv", 3)
                wk = load_w(dwq_d.ap()[i, 1], "wqkv", 3)
                wv = load_w(dwq_d.ap()[i, 2], "wqkv", 3)
                wo = load_w(dwq_d.ap()[i, 3], "wo", 2)
                X = attention(X, X, wq, wk, wv, wo, tgt_bias, X)
                X = ln_inplace(X)
                wq = load_w(dwq_d.ap()[i, 4], "wqkv", 3)
                wk = load_w(dwq_d.ap()[i, 5], "wqkv", 3)
                wv = load_w(dwq_d.ap()[i, 6], "wqkv", 3)
                wo = load_w(dwq_d.ap()[i, 7], "wo", 2)
                X = attention(X, memb, wq, wk, wv, wo, src_bias, X)
                X = ln_inplace(X)
                X = ffn(X, dwf1_d.ap()[i], dwf2_d.ap()[i], X)
                X = ln_inplace(X)

            # ---------------- tied vocab head ----------------
            vb = 0
            while vb < V:
                nb = min(256, V - vb)
                tkt = wp.tile([P, KO, 256], F32R, tag="tokT", bufs=3)
                nc.sync.dma_start(tkt[:, :, :nb], r3(tokT_d.ap()[:, vb:vb + nb]))
                for j in range(nb // P):
                    pst = ps.tile([P, L], FP32, tag="proj", bufs=4)
                    for ko in range(KO):
                        nc.tensor.matmul(pst[:], tkt[:, ko, j * P:(j + 1) * P],
                                         X[:, ko, :], start=(ko == 0), stop=(ko == KO - 1))
                    ot = ap_.tile([P, L], FP32, tag="lout")
                    nc.any.tensor_copy(ot[:], pst[:])
                    nc.sync.dma_start(out_d.ap()[vb + j * P: vb + (j + 1) * P, :], ot[:])
                vb += nb

    nc.compile()
    return nc


def _get_module():
    if "nc" not in _MODULE_CACHE:
        _MODULE_CACHE["nc"] = _build_module()
    return _MODULE_CACHE["nc"]


_QSAMPLE_SCRIPT = r"""
import sys
import numpy as np
import jax, jax.numpy as jnp
T_MAX, MASK_ID, COS_S = 1000, 4, 0.008
d = np.load(sys.argv[1])
t, tgt = d["t"], d["tgt"]
tf = jnp.asarray(t).astype(jnp.float32)
alpha_bar = jnp.cos(((tf / T_MAX) + COS_S) / (1.0 + COS_S) * (jnp.pi / 2)) ** 2
mask_prob = 1.0 - alpha_bar
u = jax.random.uniform(jax.random.key(42), tgt.shape)
x_t_ids = jnp.where(u < mask_prob[:, None], MASK_ID, jnp.asarray(tgt))
h = tf[:, None] @ jnp.asarray(d["t1w"]) + jnp.asarray(d["t1b"])
t_emb = jax.nn.silu(h) @ jnp.asarray(d["t2w"]) + jnp.asarray(d["t2b"])
np.savez(sys.argv[2], mask_prob=np.asarray(mask_prob),
         x_t_ids=np.asarray(x_t_ids), t_emb=np.asarray(t_emb))
"""


def _host_prep(src, tgt, t, params):
    """All cheap index/elementwise prep, replicating reference numerics.

    The q_sample randomness (jax threefry) is backend-dependent: the axon
    plugin returns different bits than XLA-CPU, so it MUST run in a CPU-only
    subprocess to match the reference."""
    import os
    import subprocess
    import sys
    import tempfile

    src = np.asarray(src)
    tgt = np.asarray(tgt)
    t = np.asarray(t)

    with tempfile.TemporaryDirectory() as td:
        fin = os.path.join(td, "in.npz")
        fout = os.path.join(td, "out.npz")
        np.savez(fin, t=t, tgt=tgt,
                 t1w=np.asarray(params["t1"]["W"]), t1b=np.asarray(params["t1"]["b"]),
                 t2w=np.asarray(params["t2"]["W"]), t2b=np.asarray(params["t2"]["b"]))
        env = dict(os.environ, JAX_PLATFORMS="cpu")
        subprocess.run([sys.executable, "-c", _QSAMPLE_SCRIPT, fin, fout],
                       check=True, env=env, capture_output=True)
        out = np.load(fout)
        mask_prob = out["mask_prob"]
        x_t_ids = out["x_t_ids"]
        t_emb = out["t_emb"]

    x_t_probs = np.broadcast_to(mask_prob[:, None], tgt.shape).astype(np.float32)

    src_tok = np.asarray(params["src_tok"])
    src_pos = np.asarray(params["src_pos"])
    tgt_tok = np.asarray(params["tgt_tok"])
    tgt_pos = np.asarray(params["tgt_pos"])

    def chk0(x):
        assert not np.any(np.asarray(x)), "expected zero vector"

    def chk1(x):
        assert np.all(np.asarray(x) == 1.0), "expected unit vector"

    def f32(x):
        return np.ascontiguousarray(np.asarray(x).astype(np.float32))

    qs = np.float32(1.0 / np.sqrt(HD))  # 0.125, exact
    ew_qkvo = np.empty((NL, 4, D, D), np.float32)
    ew_f1 = np.empty((NL, D, DF), np.float32)
    ew_f2 = np.empty((NL, DF, D), np.float32)
    for i, blk in enumerate(params["enc"]):
        m = blk["mha"]
        ew_qkvo[i, 0] = f32(np.asarray(m["q"]["W"]) * qs)
        ew_qkvo[i, 1] = f32(m["k"]["W"])
        ew_qkvo[i, 2] = f32(m["v"]["W"])
        ew_qkvo[i, 3] = f32(m["o"]["W"])
        ew_f1[i] = f32(blk["ff1"]["W"])
        ew_f2[i] = f32(blk["ff2"]["W"])
        for lin in (m["q"], m["k"], m["v"], m["o"], blk["ff1"], blk["ff2"]):
            chk0(lin["b"])
        for ln in (blk["ln1"], blk["ln2"]):
            chk1(ln["g"]); chk0(ln["b"])
    dw_qkvo = np.empty((NL, 8, D, D), np.float32)
    dw_f1 = np.empty((NL, D, DF), np.float32)
    dw_f2 = np.empty((NL, DF, D), np.float32)
    for i, blk in enumerate(params["dec"]):
        for j, m in enumerate((blk["sa"], blk["ca"])):
            dw_qkvo[i, 4 * j + 0] = f32(np.asarray(m["q"]["W"]) * qs)
            dw_qkvo[i, 4 * j + 1] = f32(m["k"]["W"])
            dw_qkvo[i, 4 * j + 2] = f32(m["v"]["W"])
            dw_qkvo[i, 4 * j + 3] = f32(m["o"]["W"])
            for lin in (m["q"], m["k"], m["v"], m["o"]):
                chk0(lin["b"])
        dw_f1[i] = f32(blk["ff1"]["W"])
        dw_f2[i] = f32(blk["ff2"]["W"])
        chk0(blk["ff1"]["b"]); chk0(blk["ff2"]["b"])
        for ln in (blk["ln1"], blk["ln2"], blk["ln3"]):
            chk1(ln["g"]); chk0(ln["b"])
    chk0(params["head_b"])
    tokT = f32(tgt_tok.T)

    in_maps = []
    for c in range(B):
        x0e = (src_tok[src[c]] + src_pos[:L]).T.astype(np.float32)
        x0d = (tgt_tok[x_t_ids[c]] + tgt_pos[:L] + t_emb[c][None, :]).T.astype(np.float32)
        vecs = np.zeros((P, 8), np.float32)
        sb = np.where(src[c] == 0, np.float32(NEG), np.float32(0.0))
        tb = np.where(tgt[c] == 0, np.float32(NEG), np.float32(0.0))
        vecs[:, 0:4] = sb.reshape(KO, P).T
        vecs[:, 4:8] = tb.reshape(KO, P).T
        in_maps.append({
            "x0e": np.ascontiguousarray(x0e),
            "x0d": np.ascontiguousarray(x0d),
            "ew_qkvo": ew_qkvo, "ew_f1": ew_f1, "ew_f2": ew_f2,
            "dw_qkvo": dw_qkvo, "dw_f1": dw_f1, "dw_f2": dw_f2,
            "tokT": tokT, "vecs": vecs,
        })
    return in_maps, x_t_probs


def kernel(src, tgt, t, params):
    from concourse.bass_utils import run_bass_kernel_spmd

    nc = _get_module()
    in_maps, x_t_probs = _host_prep(src, tgt, t, params)
    res = run_bass_kernel_spmd(nc, in_maps, core_ids=list(range(B)))
    logits = np.empty((B, L, V), np.float32)
    for c in range(B):
        logits[c] = res.results[c]["logitsT"].T
    return logits, x_t_probs


# revision 16
# speedup vs baseline: 1.1313x; 1.0005x over previous
"""Trainium2 Bass kernel for nn_D3PMCrossAttention (encoder-decoder diffusion
transformer, D=512 H=8 DF=2048 NL=6+6 V=16000 L=512 B=8).

Strategy: pure data-parallel over batch (1 element per NeuronCore, 8 cores,
no collectives). Activations live transposed in SBUF as [D(partitions), L(free)]
so every linear is matmul(lhsT=W[d_in,d_out], rhs=X^T) with weights used as
stored.

Precision: every matmul runs in float32r - full fp32 data processed at
1 cycle/row on the PE (same rate as bf16 for moving dim >= 256), ~1.5e-4
per-matmul error. A single PE dtype also avoids the fp32r/bf16 mode-switch
hazard (alternating dtypes was observed to corrupt results and even hang the
PE). PSUM accumulation, layernorm and softmax arithmetic are fp32. This is
deliberately the memory-bound regime: ~245 MB of fp32 weights stream per
core per call, overlapped with compute.

Tricks:
- softmax: scores built transposed [k, q] so the padding-mask bias is a
  per-partition scalar folded into the Exp activation; denominators come from
  a 65th all-ones column appended to V in the attn@V matmul; the reciprocal
  row is broadcast across partitions on the (otherwise idle) GPSIMD.
- layernorm: sum/sumsq over the feature dim (partitions) via an all-ones
  128x128 lhsT matmul, which reduces AND broadcasts to all partitions in one
  PE op.
- 1/sqrt(HD) pre-folded into Wq on the host (exact: x0.125).
- all linear biases / ln g,b / head_b are structurally zero/one in this
  problem (asserted on host) and skipped on device.
"""
import numpy as np

D, H, DF, NL, V, LMAX, B = 512, 8, 2048, 6, 16000, 512, 8
T_MAX, MASK_ID, COS_S = 1000, 4, 0.008
HD = D // H          # 64
P = 128
KO = D // P          # 4 chunks of the feature dim
KF = DF // P         # 16 chunks of the ff dim
L = LMAX
NEG = -60.0          # mask bias (exp(-60) ~ 9e-27, close enough to -inf)

_MODULE_CACHE = {}


def _build_module():
    import concourse.bacc as bacc
    import concourse.mybir as mybir
    import concourse.tile as tile

    FP32 = mybir.dt.float32
    F32R = mybir.dt.float32r
    AF = mybir.ActivationFunctionType

    nc = bacc.Bacc("TRN2", target_bir_lowering=False, debug=False, num_devices=8)

    # ---------------- DRAM I/O ----------------
    x0e_d = nc.dram_tensor("x0e", [D, L], F32R, kind="ExternalInput")
    x0d_d = nc.dram_tensor("x0d", [D, L], F32R, kind="ExternalInput")
    ewq_d = nc.dram_tensor("ew_qkvo", [NL, 4, D, D], F32R, kind="ExternalInput")
    ewf1_d = nc.dram_tensor("ew_f1", [NL, D, DF], F32R, kind="ExternalInput")
    ewf2_d = nc.dram_tensor("ew_f2", [NL, DF, D], F32R, kind="ExternalInput")
    dwq_d = nc.dram_tensor("dw_qkvo", [NL, 8, D, D], F32R, kind="ExternalInput")
    dwf1_d = nc.dram_tensor("dw_f1", [NL, D, DF], F32R, kind="ExternalInput")
    dwf2_d = nc.dram_tensor("dw_f2", [NL, DF, D], F32R, kind="ExternalInput")
    tokT_d = nc.dram_tensor("tokT", [D, V], F32R, kind="ExternalInput")
    vecs_d = nc.dram_tensor("vecs", [P, 8], FP32, kind="ExternalInput")
    out_d = nc.dram_tensor("logitsT", [V, L], FP32, kind="ExternalOutput")

    def r3(ap):  # [D_in, N] dram -> [128, ko, N]
        return ap.rearrange("(ko p) n -> p ko n", p=P)

    with tile.TileContext(nc) as tc:
        with tc.tile_pool(name="const", bufs=1) as cst, \
             tc.tile_pool(name="w", bufs=2) as wp, \
             tc.tile_pool(name="act", bufs=2) as ap_, \
             tc.tile_pool(name="ps", bufs=2, space="PSUM") as ps:

            # constants
            onesf = cst.tile([P, P], FP32)
            nc.vector.memset(onesf[:], 1.0)
            ones_r = cst.tile([P, P], F32R)
            nc.vector.tensor_copy(ones_r[:], onesf[:])
            zcol = cst.tile([P, 2], FP32)
            nc.vector.memset(zcol[:, 0:1], 0.0)
            nc.vector.memset(zcol[:, 1:2], 1e-5)
            bias0 = zcol[:, 0:1]
            biaseps = zcol[:, 1:2]
            vecs = cst.tile([P, 8], FP32)
            nc.sync.dma_start(vecs[:], vecs_d.ap())
            src_bias = vecs[:, 0:4]   # [128, 4] per k-chunk columns
            tgt_bias = vecs[:, 4:8]

            def load_w(dram_ap, tag, bufs):
                t = wp.tile([P, KO, D], F32R, tag=tag, bufs=bufs)
                nc.sync.dma_start(t[:], r3(dram_ap))
                return t

            def ln_inplace(X):
                """X (f32r [P,KO,L]) -> Xn (f32r) layernormed over d.

                Minimal serial chain: var*D = SS - S^2/D computed as
                TT(pSS - Square(pS * 1/sqrt(D))); sd = Sqrt(varD/D + eps)
                with the 1/D folded into the activation scale. The mean
                shift is applied as Xn = X*rstd - (pS*rstd)/D."""
                Xsq = ap_.tile([P, KO, L], F32R, tag="Xsq", bufs=1)
                for mo in range(KO):
                    nc.scalar.activation(Xsq[:, mo, :], X[:, mo, :], AF.Square,
                                         bias=bias0)
                pS = ps.tile([P, L], FP32, tag="attnv")
                pSS = ps.tile([P, L], FP32, tag="attnv")
                for ko in range(KO):
                    nc.tensor.matmul(pS[:], ones_r[:], X[:, ko, :],
                                     start=(ko == 0), stop=(ko == KO - 1))
                for ko in range(KO):
                    nc.tensor.matmul(pSS[:], ones_r[:], Xsq[:, ko, :],
                                     start=(ko == 0), stop=(ko == KO - 1))
                ps2 = ap_.tile([P, L], FP32, tag="stat", bufs=6)
                nc.scalar.activation(ps2[:], pS[:], AF.Square, bias=bias0,
                                     scale=float(1.0 / np.sqrt(D)))
                varD = ap_.tile([P, L], FP32, tag="stat", bufs=6)
                nc.vector.tensor_sub(varD[:], pSS[:], ps2[:])
                sd = ap_.tile([P, L], FP32, tag="stat", bufs=6)
                nc.scalar.activation(sd[:], varD[:], AF.Sqrt, bias=biaseps,
                                     scale=float(1.0 / D))
                rstd = ap_.tile([P, L], FP32, tag="stat", bufs=6)
                nc.vector.reciprocal(rstd[:], sd[:])
                cbD = ap_.tile([P, L], FP32, tag="stat", bufs=6)
                nc.vector.tensor_mul(cbD[:], pS[:], rstd[:])
                c_b = ap_.tile([P, L], FP32, tag="stat", bufs=6)
                nc.vector.tensor_scalar_mul(c_b[:], cbD[:], 1.0 / D)
                Xn = ap_.tile([P, KO, L], F32R, tag="X")
                for mo in range(KO):
                    nc.vector.tensor_mul(Xn[:, mo, :], X[:, mo, :], rstd[:])
                    nc.vector.tensor_sub(Xn[:, mo, :], Xn[:, mo, :], c_b[:])
                return Xn

            def attention(Xq, KV, wq, wk, wv, wo, mask_cols, X_res):
                """MHA block, all f32r. Returns X_res + attn_out (f32r)."""
                QT = ap_.tile([P, KO, L], F32R, tag="QT", bufs=1)
                KT = ap_.tile([P, KO, L], F32R, tag="KT", bufs=1)
                for dst, w, src_ in ((QT, wq, Xq), (KT, wk, KV)):
                    for mo in range(KO):
                        pst = ps.tile([P, L], FP32, tag="proj", bufs=4)
                        for ko in range(KO):
                            nc.tensor.matmul(pst[:], w[:, ko, mo * P:(mo + 1) * P],
                                             src_[:, ko, :], start=(ko == 0),
                                             stop=(ko == KO - 1))
                        nc.any.tensor_copy(dst[:, mo, :], pst[:])
                # V in normal layout [k, d] with per-head ones column (65-wide)
                Vpp = ap_.tile([P, KO, H * 65], F32R, tag="Vpp", bufs=1)
                nc.vector.tensor_copy(
                    Vpp[:].rearrange("p m (h c) -> p m h c", c=65)[:, :, :, 64],
                    onesf[:, None, 0:8].to_broadcast([P, KO, 8]))
                for mo in range(KO):
                    pst = ps.tile([P, L], FP32, tag="proj", bufs=4)
                    for ko in range(KO):
                        nc.tensor.matmul(pst[:], KV[:, ko, mo * P:(mo + 1) * P],
                                         wv[:, ko, :], start=(ko == 0), stop=(ko == KO - 1))
                    nc.any.tensor_copy(
                        Vpp[:, mo, :].rearrange("p (h c) -> p h c", c=65)[:, :, 0:64],
                        pst[:])
                # per-head attention
                AOT = ap_.tile([P, KO, L], F32R, tag="AOT", bufs=1)
                for h in range(H):
                    koh, p0 = h // 2, (h % 2) * 64
                    pso = ps.tile([65, L], FP32, tag="attnv")
                    for kc in range(KO):
                        pss = ps.tile([P, L], FP32, tag="pssc")
                        nc.tensor.matmul(pss[:],
                                         KT[p0:p0 + 64, koh, kc * P:(kc + 1) * P],
                                         QT[p0:p0 + 64, koh, :], start=True, stop=True)
                        expT = ap_.tile([P, L], F32R, tag="expT", bufs=6)
                        nc.scalar.activation(expT[:], pss[:], AF.Exp,
                                             bias=mask_cols[:, kc:kc + 1])
                        nc.tensor.matmul(pso[:],
                                         Vpp[:, kc, h * 65:(h + 1) * 65],
                                         expT[:], start=(kc == 0), stop=(kc == KO - 1))
                    r_sb = ap_.tile([1, L], FP32, tag="stat", bufs=6)
                    nc.vector.reciprocal(r_sb[:], pso[64:65, :])
                    rbc = ap_.tile([64, L], FP32, tag="rbc")
                    nc.gpsimd.partition_broadcast(rbc[:], r_sb[:])
                    nc.vector.tensor_mul(AOT[p0:p0 + 64, koh, :], pso[0:64, :], rbc[:])
                # output projection + residual
                Xn = ap_.tile([P, KO, L], F32R, tag="X")
                for mo in range(KO):
                    pst = ps.tile([P, L], FP32, tag="proj", bufs=4)
                    for ko in range(KO):
                        nc.tensor.matmul(pst[:], wo[:, ko, mo * P:(mo + 1) * P],
                                         AOT[:, ko, :], start=(ko == 0), stop=(ko == KO - 1))
                    nc.vector.tensor_add(Xn[:, mo, :], pst[:], X_res[:, mo, :])
                return Xn

            def ffn(Xn, f1_ap, f2_ap, X_res):
                """f32r FFN with d_ff processed in 4 streamed quarters."""
                psums = [ps.tile([P, L], FP32, tag="proj", bufs=4, name=f"ffn_ps{_m}")
                         for _m in range(KO)]
                for qd in range(4):
                    w1q = wp.tile([P, KO, 512], F32R, tag="w1q", bufs=2)
                    nc.sync.dma_start(w1q[:], f1_ap[:, qd * 512:(qd + 1) * 512]
                                      .rearrange("(ko p) n -> p ko n", p=P))
                    w2q = wp.tile([P, KO, D], F32R, tag="w2q", bufs=2)
                    nc.sync.dma_start(w2q[:], f2_ap[qd * 512:(qd + 1) * 512, :]
                                      .rearrange("(ko p) n -> p ko n", p=P))
                    HTq = ap_.tile([P, KO, L], F32R, tag="HTq")
                    for m2 in range(KO):
                        pst = ps.tile([P, L], FP32, tag="pssc")
                        for ko in range(KO):
                            nc.tensor.matmul(pst[:], w1q[:, ko, m2 * P:(m2 + 1) * P],
                                             Xn[:, ko, :], start=(ko == 0), stop=(ko == KO - 1))
                        nc.scalar.activation(HTq[:, m2, :], pst[:], AF.Relu, bias=bias0)
                    for mo in range(KO):
                        for kq in range(KO):
                            nc.tensor.matmul(psums[mo][:],
                                             w2q[:, kq, mo * P:(mo + 1) * P],
                                             HTq[:, kq, :],
                                             start=(qd == 0 and kq == 0),
                                             stop=(qd == 3 and kq == KO - 1))
                Xo = ap_.tile([P, KO, L], F32R, tag="X")
                for mo in range(KO):
                    nc.vector.tensor_add(Xo[:, mo, :], psums[mo][:], X_res[:, mo, :])
                return Xo

            # ---------------- encoder ----------------
            X = ap_.tile([P, KO, L], F32R, tag="X")
            nc.sync.dma_start(X[:], r3(x0e_d.ap()))
            for i in range(NL):
                wq = load_w(ewq_d.ap()[i, 0], "wqkv", 3)
                wk = load_w(ewq_d.ap()[i, 1], "wqkv", 3)
                wv = load_w(ewq_d.ap()[i, 2], "wqkv", 3)
                wo = load_w(ewq_d.ap()[i, 3], "wo", 2)
                X = attention(X, X, wq, wk, wv, wo, src_bias, X)
                X = ln_inplace(X)
                X = ffn(X, ewf1_d.ap()[i], ewf2_d.ap()[i], X)
                X = ln_inplace(X)
            memb = cst.tile([P, KO, L], F32R, tag="memb")
            nc.vector.tensor_copy(memb[:], X[:])

            # ---------------- decoder ----------------
            X = ap_.tile([P, KO, L], F32R, tag="X")
            nc.sync.dma_start(X[:], r3(x0d_d.ap()))
            for i in range(NL):
                wq = load_w(dwq_d.ap()[i, 0], "wqkv", 3)
                wk = load_w(dwq_d.ap()[i, 1], "wqkv", 3)
                wv = load_w(dwq_d.ap()[i, 2], "wqkv", 3)
                wo = load_w(dwq_d.ap()[i, 3], "wo", 2)
                X = attention(X, X, wq, wk, wv, wo, tgt_bias, X)
                X = ln_inplace(X)
                wq = load_w(dwq_d.ap()[i, 4], "wqkv", 3)
                wk = load_w(dwq_d.ap()[i, 5], "wqkv", 3)
                wv = load_w(dwq_d.ap()[i, 6], "wqkv", 3)
                wo = load_w(dwq_d.ap()[i, 7], "wo", 2)
                X = attention(X, memb, wq, wk, wv, wo, src_bias, X)
                X = ln_inplace(X)
                X = ffn(X, dwf1_d.ap()[i], dwf2_d.ap()[i], X)
                X = ln_inplace(X)

            # ---------------- tied vocab head ----------------
            vb = 0
            while vb < V:
                nb = min(256, V - vb)
                tkt = wp.tile([P, KO, 256], F32R, tag="tokT", bufs=3)
                nc.sync.dma_start(tkt[:, :, :nb], r3(tokT_d.ap()[:, vb:vb + nb]))
                for j in range(nb // P):
                    pst = ps.tile([P, L], FP32, tag="proj", bufs=4)
                    for ko in range(KO):
                        nc.tensor.matmul(pst[:], tkt[:, ko, j * P:(j + 1) * P],
                                         X[:, ko, :], start=(ko == 0), stop=(ko == KO - 1))
                    ot = ap_.tile([P, L], FP32, tag="lout")
                    nc.any.tensor_copy(ot[:], pst[:])
                    nc.sync.dma_start(out_d.ap()[vb + j * P: vb + (j + 1) * P, :], ot[:])
                vb += nb

    nc.compile()
    return nc


def _get_module():
    if "nc" not in _MODULE_CACHE:
        _MODULE_CACHE["nc"] = _build_module()
    return _MODULE_CACHE["nc"]


_QSAMPLE_SCRIPT = r"""
import sys
import numpy as np
import jax, jax.numpy as jnp
T_MAX, MASK_ID, COS_S = 1000, 4, 0.008
d = np.load(sys.argv[1])
t, tgt = d["t"], d["tgt"]
tf = jnp.asarray(t).astype(jnp.float32)
alpha_bar = jnp.cos(((tf / T_MAX) + COS_S) / (1.0 + COS_S) * (jnp.pi / 2)) ** 2
mask_prob = 1.0 - alpha_bar
u = jax.random.uniform(jax.random.key(42), tgt.shape)
x_t_ids = jnp.where(u < mask_prob[:, None], MASK_ID, jnp.asarray(tgt))
h = tf[:, None] @ jnp.asarray(d["t1w"]) + jnp.asarray(d["t1b"])
t_emb = jax.nn.silu(h) @ jnp.asarray(d["t2w"]) + jnp.asarray(d["t2b"])
np.savez(sys.argv[2], mask_prob=np.asarray(mask_prob),
         x_t_ids=np.asarray(x_t_ids), t_emb=np.asarray(t_emb))
"""


def _host_prep(src, tgt, t, params):
    """All cheap index/elementwise prep, replicating reference numerics.

    The q_sample randomness (jax threefry) is backend-dependent: the axon
    plugin returns different bits than XLA-CPU, so it MUST run in a CPU-only
    subprocess to match the reference."""
    import os
    import subprocess
    import sys
    import tempfile

    src = np.asarray(src)
    tgt = np.asarray(tgt)
    t = np.asarray(t)

    with tempfile.TemporaryDirectory() as td:
        fin = os.path.join(td, "in.npz")
        fout = os.path.join(td, "out.npz")
        np.savez(fin, t=t, tgt=tgt,
                 t1w=np.asarray(params["t1"]["W"]), t1b=np.asarray(params["t1"]["b"]),
                 t2w=np.asarray(params["t2"]["W"]), t2b=np.asarray(params["t2"]["b"]))
        env = dict(os.environ, JAX_PLATFORMS="cpu")
        subprocess.run([sys.executable, "-c", _QSAMPLE_SCRIPT, fin, fout],
                       check=True, env=env, capture_output=True)
        out = np.load(fout)
        mask_prob = out["mask_prob"]
        x_t_ids = out["x_t_ids"]
        t_emb = out["t_emb"]

    x_t_probs = np.broadcast_to(mask_prob[:, None], tgt.shape).astype(np.float32)

    src_tok = np.asarray(params["src_tok"])
    src_pos = np.asarray(params["src_pos"])
    tgt_tok = np.asarray(params["tgt_tok"])
    tgt_pos = np.asarray(params["tgt_pos"])

    def chk0(x):
        assert not np.any(np.asarray(x)), "expected zero vector"

    def chk1(x):
        assert np.all(np.asarray(x) == 1.0), "expected unit vector"

    def f32(x):
        return np.ascontiguousarray(np.asarray(x).astype(np.float32))

    qs = np.float32(1.0 / np.sqrt(HD))  # 0.125, exact
    ew_qkvo = np.empty((NL, 4, D, D), np.float32)
    ew_f1 = np.empty((NL, D, DF), np.float32)
    ew_f2 = np.empty((NL, DF, D), np.float32)
    for i, blk in enumerate(params["enc"]):
        m = blk["mha"]
        ew_qkvo[i, 0] = f32(np.asarray(m["q"]["W"]) * qs)
        ew_qkvo[i, 1] = f32(m["k"]["W"])
        ew_qkvo[i, 2] = f32(m["v"]["W"])
        ew_qkvo[i, 3] = f32(m["o"]["W"])
        ew_f1[i] = f32(blk["ff1"]["W"])
        ew_f2[i] = f32(blk["ff2"]["W"])
        for lin in (m["q"], m["k"], m["v"], m["o"], blk["ff1"], blk["ff2"]):
            chk0(lin["b"])
        for ln in (blk["ln1"], blk["ln2"]):
            chk1(ln["g"]); chk0(ln["b"])
    dw_qkvo = np.empty((NL, 8, D, D), np.float32)
    dw_f1 = np.empty((NL, D, DF), np.float32)
    dw_f2 = np.empty((NL, DF, D), np.float32)
    for i, blk in enumerate(params["dec"]):
        for j, m in enumerate((blk["sa"], blk["ca"])):
            dw_qkvo[i, 4 * j + 0] = f32(np.asarray(m["q"]["W"]) * qs)
            dw_qkvo[i, 4 * j + 1] = f32(m["k"]["W"])
            dw_qkvo[i, 4 * j + 2] = f32(m["v"]["W"])
            dw_qkvo[i, 4 * j + 3] = f32(m["o"]["W"])
            for lin in (m["q"], m["k"], m["v"], m["o"]):
                chk0(lin["b"])
        dw_f1[i] = f32(blk["ff1"]["W"])
        dw_f2[i] = f32(blk["ff2"]["W"])
        chk0(blk["ff1"]["b"]); chk0(blk["ff2"]["b"])
        for ln in (blk["ln1"], blk["ln2"], blk["ln3"]):
            chk1(ln["g"]); chk0(ln["b"])
    chk0(params["head_b"])
    tokT = f32(tgt_tok.T)

    in_maps = []
    for c in range(B):
        x0e = (src_tok[src[c]] + src_pos[:L]).T.astype(np.float32)
        x0d = (tgt_tok[x_t_ids[c]] + tgt_pos[:L] + t_emb[c][None, :]).T.astype(np.float32)
        vecs = np.zeros((P, 8), np.float32)
        sb = np.where(src[c] == 0, np.float32(NEG), np.float32(0.0))
        tb = np.where(tgt[c] == 0, np.float32(NEG), np.float32(0.0))
        vecs[:, 0:4] = sb.reshape(KO, P).T
        vecs[:, 4:8] = tb.reshape(KO, P).T
        in_maps.append({
            "x0e": np.ascontiguousarray(x0e),
            "x0d": np.ascontiguousarray(x0d),
            "ew_qkvo": ew_qkvo, "ew_f1": ew_f1, "ew_f2": ew_f2,
            "dw_qkvo": dw_qkvo, "dw_f1": dw_f1, "dw_f2": dw_f2,
            "tokT": tokT, "vecs": vecs,
        })
    return in_maps, x_t_probs


def kernel(src, tgt, t, params):
    from concourse.bass_utils import run_bass_kernel_spmd

    nc = _get_module()
    in_maps, x_t_probs = _host_prep(src, tgt, t, params)
    try:
        res = run_bass_kernel_spmd(nc, in_maps, core_ids=list(range(B)))
    except Exception:
        res = run_bass_kernel_spmd(nc, in_maps, core_ids=list(range(B)))
    logits = np.empty((B, L, V), np.float32)
    for c in range(B):
        logits[c] = res.results[c]["logitsT"].T
    return logits, x_t_probs
